# revision 17
# baseline (speedup 1.0000x reference)
"""Trainium2 Bass kernel for nn_Aggregate (GNN message passing / COO SpMM + Linear).

Computes: y = segment_sum(edge_val[:,None] * x[edge_col], edge_row, N) @ W.T

Strategy (8 NeuronCores, SPMD):
  - Shard destination nodes across the 8 cores (N/8 rows each); sort edges by
    destination on the host and route each edge to the core owning its dest row.
  - Replicate x in every core's HBM. Each core gathers x[edge_col] rows for its
    edges with gpsimd dma_gather (int16 indices -> x is split into row banks of
    <=32k rows; edges are grouped by (dest window, bank) with each group padded
    to a multiple of 128).
  - Segment-sum on the TensorEngine: for each window of 128 destination rows,
    accumulate sum_e onehot(row)*val * x[col] into PSUM as a sequence of
    128-edge matmuls: lhsT = sel[e,r] = (iota[r]==row_local[e])*val[e] (one
    fused DVE tensor_scalar op per chunk), rhs = gathered features [e,d].
  - Per window: y_win @ W.T via PE transpose + matmul with W.T; DMA out.

Self-contained: numpy + the concourse/bass stack at /opt/trn_rl_repo.
"""

import os
import sys

for _p in ("/opt/trn_rl_repo",):
    if _p not in sys.path and os.path.isdir(_p):
        sys.path.insert(0, _p)

import numpy as np

import concourse.bass as bass
import concourse.mybir as mybir
import concourse.tile as tile
from concourse import bacc
from concourse.bass_utils import run_bass_kernel_spmd

P = 128
NCORES = 8
MAX_BANK = 32000  # int16 index headroom
F32 = mybir.dt.float32
I16 = mybir.dt.int16

# Populated by the most recent kernel() call (test harness reads these).
LAST_RESULTS = None


def _install_ntff_shim():
    """The agent image's `antenv` lacks `axon_hooks`; provide it so
    run_bass_kernel_spmd(trace=True) can reach the NTFF profiler."""
    import types

    if "antenv.axon_hooks" in sys.modules:
        return
    try:
        from trn_agent_boot.trn_boot import _ntff_profile_via_ctypes
    except ImportError:
        return
    hook = _ntff_profile_via_ctypes("/opt/axon/libaxon_pjrt.so")
    mod = types.ModuleType("antenv.axon_hooks")
    mod.get_axon_ntff_profile_hook = lambda: hook
    mod.set_axon_ntff_profile_hook = lambda h: None
    sys.modules["antenv.axon_hooks"] = mod
    # the artifact upload wants a remote bucket that is unreachable here
    import concourse.bass_utils as _bu

    _bu.upload_artifacts = lambda tmpdir: f"local:{tmpdir}"


def _preprocess(edge_row, edge_col, edge_val, n_nodes):
    """Sort edges by dest, shard by dest across cores, group each core's
    window edges by source bank, pad each (window, bank) group to a multiple
    of 128 (shared counts across cores for SPMD).

    Returns:
      wrap_idx: list of NB arrays [P, len_b//16] int16 (dma_gather index wrap)
      rows_t, vals_t: [NCORES, P, K_total] f32 (chunk-transposed metadata)
      chunk_bank, chunk_slot: [K_total] int arrays: which bank stream + slot
        each global chunk reads from
      win_chunks: [n_win] number of chunks per window
      n_win, shard, n_banks, bank_size
    """
    shard = n_nodes // NCORES
    n_win = (shard + P - 1) // P
    n_banks = max(1, -(-n_nodes // MAX_BANK))
    bank_size = -(-n_nodes // n_banks)
    NB = n_banks

    row = np.asarray(edge_row).astype(np.int64).ravel()
    col = np.asarray(edge_col).astype(np.int64).ravel()
    val = np.asarray(edge_val).astype(np.float32).ravel()

    core_of = row // shard
    win_of = (row - core_of * shard) // P
    row_local = (row - core_of * shard - win_of * P).astype(np.float32)
    bank_of = col // bank_size

    # group key per edge: (core, win, bank)
    key = (core_of * n_win + win_of) * NB + bank_of
    counts = np.bincount(key, minlength=NCORES * n_win * NB).reshape(
        NCORES, n_win, NB
    )
    # shared chunk counts: max over cores
    chunks_wb = (counts.max(axis=0) + P - 1) // P  # [n_win, NB]
    # ensure every window has at least one chunk
    empty = chunks_wb.sum(axis=1) == 0
    chunks_wb[empty, 0] = 1

    win_chunks = chunks_wb.sum(axis=1)  # [n_win]
    k_total = int(win_chunks.sum())

    # global chunk order: w-major, then bank
    flat_chunks = chunks_wb.ravel()  # [(w,b)] -> count
    grp_chunk_base = np.concatenate([[0], np.cumsum(flat_chunks)])[:-1]  # global
    # per-bank slot base for each (w,b) group
    bank_len = chunks_wb.sum(axis=0)  # [NB] chunks per bank stream
    grp_bank_base = np.zeros((n_win, NB), dtype=np.int64)
    grp_bank_base[1:] = np.cumsum(chunks_wb[:-1], axis=0)

    # chunk -> (bank, slot) mapping
    chunk_bank = np.zeros(k_total, dtype=np.int64)
    chunk_slot = np.zeros(k_total, dtype=np.int64)
    for w in range(n_win):
        for b in range(NB):
            n = chunks_wb[w, b]
            if n == 0:
                continue
            g0 = grp_chunk_base[w * NB + b]
            chunk_bank[g0 : g0 + n] = b
            chunk_slot[g0 : g0 + n] = grp_bank_base[w, b] + np.arange(n)

    rows_t = np.zeros((NCORES, P, k_total), dtype=np.float32)
    vals_t = np.zeros((NCORES, P, k_total), dtype=np.float32)
    wrap_idx = [
        np.zeros((NCORES, int(bank_len[b]) * P), dtype=np.int16) for b in range(NB)
    ]

    order = np.argsort(key, kind="stable")
    key_s = key[order]
    # position of each edge within its (core,win,bank) group
    grp_change = np.concatenate([[True], key_s[1:] != key_s[:-1]])
    grp_start = np.flatnonzero(grp_change)
    rep = np.diff(np.concatenate([grp_start, [key_s.shape[0]]]))
    pos_in_grp = np.arange(key_s.shape[0]) - np.repeat(grp_start, rep)

    core_s = key_s // (n_win * NB)
    wb_s = key_s % (n_win * NB)
    w_s = wb_s // NB
    b_s = wb_s % NB
    gchunk = grp_chunk_base[wb_s] + pos_in_grp // P  # global chunk id
    p_s = pos_in_grp % P
    bslot = grp_bank_base[w_s, b_s] + pos_in_grp // P  # bank-stream slot
    q_s = bslot * P + p_s  # bank-stream position

    col_rebased = (col[order] - b_s * bank_size).astype(np.int16)
    rl_s = row_local[order]
    v_s = val[order]

    for c in range(NCORES):
        m = core_s == c
        rows_t[c, p_s[m], gchunk[m]] = rl_s[m]
        vals_t[c, p_s[m], gchunk[m]] = v_s[m]
        for b in range(NB):
            mb = m & (b_s == b)
            wrap_idx[b][c, q_s[mb]] = col_rebased[mb]

    # packed wrap layout: ONE [P, max_cols] tile; bank b (served by queue b on
    # Q7 cores 2b/2b+1 = partitions 32b..32b+31) carries its wrap in that band,
    # replicated to both 16-row groups of the band.
    max_cols = max(int(bank_len[b]) * P // 16 for b in range(NB))
    wrap_sb = np.zeros((NCORES, P, max_cols), dtype=np.int16)
    for b in range(NB):
        lb = int(bank_len[b]) * P
        if lb == 0:
            continue
        band = (b % 4) * 32
        for c in range(NCORES):
            blk = wrap_idx[b][c].reshape(lb // 16, 16).T  # [16, lb/16]
            wrap_sb[c, band : band + 16, : lb // 16] = blk
            wrap_sb[c, band + 16 : band + 32, : lb // 16] = blk

    return (
        wrap_sb,
        rows_t,
        vals_t,
        chunk_bank,
        chunk_slot,
        bank_len.astype(np.int64),
        win_chunks.astype(np.int64),
        n_win,
        shard,
        NB,
        bank_size,
    )


def _build(n_nodes, k_total, win_chunks, chunk_bank, chunk_slot, bank_len,
           n_banks, bank_size, gather_batch):
    """Build the SPMD Bass program (same program on all 8 cores)."""
    n_win = len(win_chunks)
    n_queues = min(4, max(1, n_banks))
    nc = bacc.Bacc("TRN2", target_bir_lowering=False, debug=False,
                   num_swdge_queues=n_queues,
                   dynamic_dma_scratch_size=int(os.environ.get("KSCRATCH", "16384")))

    x_d = nc.dram_tensor("x", [n_nodes, P], F32, kind="ExternalInput")
    row_d = nc.dram_tensor("rowl", [P, k_total], F32, kind="ExternalInput")
    val_d = nc.dram_tensor("val", [P, k_total], F32, kind="ExternalInput")
    wt_d = nc.dram_tensor("wt", [P, P], F32, kind="ExternalInput")
    iota_d = nc.dram_tensor("iota", [P, P], F32, kind="ExternalInput")
    iotab_d = nc.dram_tensor("iotab", [P, 16 * P], F32, kind="ExternalInput")
    ident_d = nc.dram_tensor("ident", [P, P], F32, kind="ExternalInput")
    max_icols = max(int(bank_len[b]) * P // 16 for b in range(n_banks))
    idx_d = nc.dram_tensor("idxp", [P, max_icols], I16, kind="ExternalInput")
    yout_d = nc.dram_tensor("y_out", [n_win * P, P], F32, kind="ExternalOutput")

    G = gather_batch

    with tile.TileContext(nc) as tc:
        with (
            tc.tile_pool(name="meta", bufs=1) as meta,
            tc.tile_pool(name="gath", bufs=int(os.environ.get("KGBUFS", "8"))) as gpool,
            tc.tile_pool(name="sel", bufs=int(os.environ.get("KSELB", "3"))) as spool,
            tc.tile_pool(name="ycopy", bufs=2) as ycopy,
            tc.tile_pool(name="ytcopy", bufs=2) as ytcopy,
            tc.tile_pool(name="ocopy", bufs=2) as ocopy,
            tc.tile_pool(name="ypsum", bufs=int(os.environ.get("KYPB", "4")), space="PSUM") as ypsum_p,
            tc.tile_pool(name="tpsum", bufs=2, space="PSUM") as tpsum_p,
            tc.tile_pool(name="opsum", bufs=2, space="PSUM") as opsum_p,
        ):
            # --- metadata + constants into SBUF
            row_sb = meta.tile([P, k_total], F32)
            val_sb = meta.tile([P, k_total], F32)
            wt_sb = meta.tile([P, P], F32)
            iota_sb = meta.tile([P, P], F32)
            iota_big = meta.tile([P, 16 * P], F32)
            id_sb = meta.tile([P, P], F32)
            nc.sync.dma_start(row_sb[:], row_d[:])
            nc.sync.dma_start(val_sb[:], val_d[:])
            nc.sync.dma_start(wt_sb[:], wt_d[:])
            nc.sync.dma_start(iota_sb[:], iota_d[:])
            nc.sync.dma_start(iota_big[:], iotab_d[:])
            nc.sync.dma_start(id_sb[:], ident_d[:])
            idx_sb = meta.tile([P, max_icols], I16)
            nc.sync.dma_start(idx_sb[:], idx_d[:])

            # per-bank gather state: current batch tile
            gtiles = [None] * n_banks

            def ensure_gather(b, slot):
                g = slot // G
                if gtiles[b] is not None and gtiles[b][0] == g:
                    return gtiles[b][1]
                width = min(G, int(bank_len[b]) - g * G)
                t = gpool.tile([P, G * P], F32, tag="gath")
                nc.gpsimd.dma_gather(
                    out_ap=t[:, : width * P].rearrange("p (k d) -> p k d", d=P),
                    in_ap=x_d[b * bank_size : min((b + 1) * bank_size, n_nodes), :],
                    idxs_ap=idx_sb[:, g * G * P // 16 : (g * G + width) * P // 16],
                    num_idxs=width * P,
                    num_idxs_reg=width * P,
                    elem_size=P,
                    single_packet=False,
                    queue_num=b % n_queues,
                )
                gtiles[b] = (g, t)
                return t

            SB = 16  # sel batch (chunks per DVE op)
            sel_tiles = {}

            def ensure_sel(k):
                t = k // SB
                if t in sel_tiles:
                    return sel_tiles[t]
                k0 = t * SB
                gb = min(SB, k_total - k0)
                st = spool.tile([P, SB * P], F32, tag="selbig")
                st3 = st[:, : gb * P].rearrange("p (g r) -> p g r", r=P)
                rb = (row_sb[:, k0 : k0 + gb]
                      .rearrange("p g -> p g ()").to_broadcast([P, gb, P]))
                vb = (val_sb[:, k0 : k0 + gb]
                      .rearrange("p g -> p g ()").to_broadcast([P, gb, P]))
                nc.vector.tensor_tensor(
                    out=st3, in0=iota_big[:, : gb * P].rearrange(
                        "p (g r) -> p g r", r=P),
                    in1=rb, op=mybir.AluOpType.is_equal)
                nc.vector.tensor_tensor(
                    out=st3, in0=st3, in1=vb, op=mybir.AluOpType.mult)
                sel_tiles.clear()
                sel_tiles[t] = st
                return st

            k = 0
            for w in range(n_win):
                ypsum = ypsum_p.tile([P, P], F32)
                nchunk = int(win_chunks[w])
                for kk in range(nchunk):
                    b = int(chunk_bank[k])
                    slot = int(chunk_slot[k])
                    gt = ensure_gather(b, slot)
                    s = slot % G
                    st = ensure_sel(k)
                    so = (k % SB) * P
                    nc.tensor.matmul(
                        out=ypsum[:],
                        lhsT=st[:, so : so + P],
                        rhs=gt[:, s * P : (s + 1) * P],
                        start=(kk == 0),
                        stop=(kk == nchunk - 1),
                    )
                    k += 1

                # --- apply W: out = y_win @ W.T  (via PE transpose)
                y_sb = ycopy.tile([P, P], F32)
                nc.scalar.copy(y_sb[:], ypsum[:])
                yt_ps = tpsum_p.tile([P, P], F32)
                nc.tensor.transpose(yt_ps[:], y_sb[:], id_sb[:])
                yt_sb = ytcopy.tile([P, P], F32)
                nc.scalar.copy(yt_sb[:], yt_ps[:])
                o_ps = opsum_p.tile([P, P], F32)
                nc.tensor.matmul(
                    out=o_ps[:], lhsT=yt_sb[:], rhs=wt_sb[:], start=True, stop=True
                )
                o_sb = ocopy.tile([P, P], F32)
                nc.vector.tensor_copy(o_sb[:], o_ps[:])
                nc.sync.dma_start(yout_d[w * P : (w + 1) * P, :], o_sb[:])

    return nc


def kernel(x, edge_row, edge_col, edge_val, W, _trace=False):
    global LAST_RESULTS
    x = np.ascontiguousarray(np.asarray(x, dtype=np.float32))
    W = np.asarray(W, dtype=np.float32)
    n_nodes = x.shape[0]
    assert x.shape[1] == P and W.shape == (P, P)

    (
        wrap_sb,
        rows_t,
        vals_t,
        chunk_bank,
        chunk_slot,
        bank_len,
        win_chunks,
        n_win,
        shard,
        n_banks,
        bank_size,
    ) = _preprocess(edge_row, edge_col, edge_val, n_nodes)
    k_total = rows_t.shape[2]
    gather_batch = min(int(os.environ.get("KGATHER", "32")), int(bank_len.max()))

    nc = _build(
        n_nodes, k_total, win_chunks, chunk_bank, chunk_slot, bank_len,
        n_banks, bank_size, gather_batch,
    )

    wt = np.ascontiguousarray(W.T)
    iota = np.tile(np.arange(P, dtype=np.float32), (P, 1))
    iotab = np.tile(np.tile(np.arange(P, dtype=np.float32), 16), (P, 1))
    ident = np.eye(P, dtype=np.float32)

    in_maps = []
    for c in range(NCORES):
        m = {
            "x": x,
            "rowl": np.ascontiguousarray(rows_t[c]),
            "val": np.ascontiguousarray(vals_t[c]),
            "wt": wt,
            "iota": iota,
            "iotab": iotab,
            "ident": ident,
        }
        m["idxp"] = np.ascontiguousarray(wrap_sb[c])
        in_maps.append(m)

    if _trace:
        _install_ntff_shim()
    if not nc.is_finalized():
        nc.finalize()
    res = run_bass_kernel_spmd(nc, in_maps, list(range(NCORES)), trace=_trace)
    LAST_RESULTS = res

    out = np.empty((n_nodes, P), dtype=np.float32)
    for c in range(NCORES):
        out[c * shard : (c + 1) * shard] = res.results[c]["y_out"][:shard]
    return out
